# revision 11
# baseline (speedup 1.0000x reference)
"""Cross-attention Bass/Tile kernel for Trainium2, sharded over 8 NeuronCores.

Problem (fixed shapes): B=2, T=2048, C=1024, H=16 heads, D=64.
    q = x_q @ Wq + bq;  kv = x_kv @ Wkv + bkv;  k, v = split(kv)
    y = softmax(q k^T / sqrt(D)) v;  out = y @ Wo + bo

Sharding: 8 cores = 2 (batch) x 4 (head groups of 4 heads, 256 channels).
Each core computes its head-group's projections + attention + a partial
output projection (its 256 rows of Wo); the host sums the 4 partials per
batch.  The v-bias and output bias are folded in exactly on the host:
    y = att@(V + 1*bv) = att@V + 1*bv   (att rows sum to 1)
    => out += bv @ Wo + bo              (added once per batch on the host)

On-chip layout (all matmuls fp32r, contraction on partitions):
 - x^T tiles produced by PE transpose (fp32 exact)
 - Q^T, K^T in [d, t] layout: head h at partitions (h%2)*64.., chunk h//2
 - V in natural [t, d] layout (lhsT of the att@V matmul)
 - S^T per (tk,tq) chunk: 2 heads row-packed into one 2-bank psum tile
 - one exp per head-pair on ScalarE, scale=1/8 folded into the affine
 - y^T accumulated in PSUM with 2 heads col-packed per bank;
   denominators via ones-lhsT matmuls col-packed at partitions 0/32/64/96
 - normalization: reciprocal + K=1 broadcast matmul + DVE multiply
 - output projection uses y^T directly as lhsT (no transpose needed)
"""

import numpy as np

B = 2
T = 2048
C = 1024
H = 16
D = 64
NCORES = 8
TPG = 4  # tensor-parallel group size (head groups)
HL = H // TPG  # heads per core = 4
CL = HL * D  # local channels = 256
P = 128

_CACHE = {}


def _build():
    import concourse.bass as bass
    import concourse.tile as tile
    from concourse import bacc, mybir
    from concourse.masks import make_identity

    f32 = mybir.dt.float32
    f32r = mybir.dt.float32r
    Exp = mybir.ActivationFunctionType.Exp

    nc = bacc.Bacc("TRN2", target_bir_lowering=False, debug=False)

    xq_d = nc.dram_tensor("xq", [T, C], f32, kind="ExternalInput")
    xkv_d = nc.dram_tensor("xkv", [T, C], f32, kind="ExternalInput")
    wq_d = nc.dram_tensor("wq", [C, CL], f32, kind="ExternalInput")
    wk_d = nc.dram_tensor("wk", [C, CL], f32, kind="ExternalInput")
    wv_d = nc.dram_tensor("wv", [C, CL], f32, kind="ExternalInput")
    wo_d = nc.dram_tensor("wo", [CL, C], f32, kind="ExternalInput")
    bq_d = nc.dram_tensor("bq", [CL], f32, kind="ExternalInput")
    bk_d = nc.dram_tensor("bk", [CL], f32, kind="ExternalInput")
    out_d = nc.dram_tensor("out", [T, C], f32, kind="ExternalOutput")

    KC = C // P  # 8 contraction chunks for the projections
    NT = T // P  # 16 token chunks of 128
    NQ = 4  # tq chunks of 512
    QW = T // NQ  # 512
    DC = CL // P  # 2 chunks of d_local

    with tile.TileContext(nc) as tc:
        with (
            tc.tile_pool(name="const", bufs=1) as const,
            tc.tile_pool(name="persist", bufs=1) as persist,
            tc.tile_pool(name="xnat", bufs=2) as xnat,
            tc.tile_pool(name="xt", bufs=1) as xtp,
            tc.tile_pool(name="ework", bufs=3) as ework,
            tc.tile_pool(name="norm1", bufs=1) as norm1,
            tc.tile_pool(name="norm2", bufs=2) as norm2,
            tc.tile_pool(name="outst", bufs=3) as outst,
            tc.tile_pool(name="dram", bufs=2, space="DRAM") as dram,
        ):
            # ---- constants / weights ----
            ident = const.tile([P, P], f32)
            make_identity(nc, ident)
            ones4_f32 = const.tile([P, HL, 1], f32)
            nc.vector.memset(ones4_f32, 1.0)

            wq_sb = const.tile([P, KC, CL], f32r)
            wk_sb = const.tile([P, KC, CL], f32r)
            wv_sb = const.tile([P, KC, CL], f32r)
            wo_sb = const.tile([P, DC, C], f32r)
            for w_sb, w_d in ((wq_sb, wq_d), (wk_sb, wk_d), (wv_sb, wv_d)):
                src = w_d.rearrange("(o p) d -> p o d", p=P).bitcast(f32r)
                for kc in range(KC):
                    nc.sync.dma_start(w_sb[:, kc, :], src[:, kc, :])
            wo_src = wo_d.rearrange("(o p) n -> p o n", p=P).bitcast(f32r)
            for dc in range(DC):
                nc.sync.dma_start(wo_sb[:, dc, :], wo_src[:, dc, :])
            bq_sb = const.tile([P, DC], f32)
            bk_sb = const.tile([P, DC], f32)
            nc.sync.dma_start(bq_sb, bq_d.rearrange("(o p) -> p o", p=P))
            nc.sync.dma_start(bk_sb, bk_d.rearrange("(o p) -> p o", p=P))

            # ---- persistent activations ----
            qt_sb = persist.tile([P, DC, T], f32r)  # Q^T  [d, t]
            kt_sb = persist.tile([P, DC, T], f32r)  # K^T  [d, t]
            v_sb = persist.tile([P, NT, HL, 66], f32r)  # V|1 [t, h, d+1]
            yt_sb = persist.tile([P, DC, T], f32r)  # y^T  [d, t] (normalized)

            # ---- stage 1: transposes + projections, per 512-token chunk ----
            with (
                tc.tile_pool(name="ps_t", bufs=4, space="PSUM") as ps_t,
                tc.tile_pool(name="ps_proj", bufs=2, space="PSUM") as ps_proj,
            ):
                for tq in range(NQ):
                    xq_t = xtp.tile([P, KC, QW], f32r, tag="xqT")
                    xkv_t = xtp.tile([P, KC, QW], f32r, tag="xkvT")
                    for ts_ in range(4):
                        tch = tq * 4 + ts_
                        x_nat = xnat.tile([P, C], f32, tag="xq_nat")
                        nc.sync.dma_start(x_nat, xq_d[tch * P : (tch + 1) * P, :])
                        kv_nat = xnat.tile([P, C], f32, tag="xkv_nat")
                        nc.sync.dma_start(kv_nat, xkv_d[tch * P : (tch + 1) * P, :])
                        for c in range(KC):
                            tp = ps_t.tile([P, P], f32, tag="tp")
                            nc.tensor.transpose(
                                tp, x_nat[:, c * P : (c + 1) * P], ident
                            )
                            nc.vector.tensor_copy(
                                xq_t[:, c, ts_ * P : (ts_ + 1) * P], tp
                            )
                            tp2 = ps_t.tile([P, P], f32, tag="tp")
                            nc.tensor.transpose(
                                tp2, kv_nat[:, c * P : (c + 1) * P], ident
                            )
                            nc.vector.tensor_copy(
                                xkv_t[:, c, ts_ * P : (ts_ + 1) * P], tp2
                            )

                    # Q^T and K^T projections: out [d_chunk 128, QW]
                    for src_t, w_sb, b_sb, dst in (
                        (xq_t, wq_sb, bq_sb, qt_sb),
                        (xkv_t, wk_sb, bk_sb, kt_sb),
                    ):
                        for dc in range(DC):
                            pp = ps_proj.tile([P, QW], f32, tag="proj")
                            for c in range(KC):
                                nc.tensor.matmul(
                                    pp,
                                    w_sb[:, c, dc * P : (dc + 1) * P],
                                    src_t[:, c, :],
                                    start=(c == 0),
                                    stop=(c == KC - 1),
                                )
                            nc.vector.tensor_scalar_add(
                                dst[:, dc, tq * QW : (tq + 1) * QW],
                                pp,
                                b_sb[:, dc : dc + 1],
                            )

                    # V projection: out [t chunk 128, CL] (no bias: folded on host)
                    for ts_ in range(4):
                        tch = tq * 4 + ts_
                        pv = ps_proj.tile([P, QW], f32, tag="proj")
                        for c in range(KC):
                            nc.tensor.matmul(
                                pv[:, :CL],
                                xkv_t[:, c, ts_ * P : (ts_ + 1) * P],
                                wv_sb[:, c, :],
                                start=(c == 0),
                                stop=(c == KC - 1),
                            )
                        nc.vector.tensor_copy(
                            v_sb[:, tch, :, 0:64],
                            pv[:, :CL].rearrange("p (h d) -> p h d", h=HL),
                        )
                        nc.vector.tensor_copy(v_sb[:, tch, :, 64:65], ones4_f32)

            # ---- stage 2: attention, per tq chunk of 512 ----
            with (
                tc.tile_pool(name="ps_s", bufs=2, space="PSUM") as ps_s,
                tc.tile_pool(name="ps_y", bufs=4, space="PSUM") as ps_y,
            ):
                for tq in range(NQ):
                    y_ps = [ps_y.tile([65, QW], f32, tag="y", name=f"y_ps{i}") for i in range(HL)]
                    for tk in range(NT):
                        for hc in range(DC):  # head pair (2*hc, 2*hc+1)
                            sp = ps_s.tile([P, 2 * QW], f32, tag="s")
                            for hh in range(2):
                                nc.tensor.matmul(
                                    sp[:, hh * QW : (hh + 1) * QW],
                                    kt_sb[
                                        hh * 64 : (hh + 1) * 64,
                                        hc,
                                        tk * P : (tk + 1) * P,
                                    ],
                                    qt_sb[
                                        hh * 64 : (hh + 1) * 64,
                                        hc,
                                        tq * QW : (tq + 1) * QW,
                                    ],
                                    start=True,
                                    stop=True,
                                    tile_position=(hh * 64, 0),
                                )
                            e2 = ework.tile([P, 2 * QW], f32r, tag="e")
                            nc.scalar.activation(e2, sp, Exp, scale=0.125)
                            for hh in range(2):
                                h = 2 * hc + hh
                                nc.tensor.matmul(
                                    y_ps[h],
                                    v_sb[:, tk, h, :65],
                                    e2[:, hh * QW : (hh + 1) * QW],
                                    start=(tk == 0),
                                    stop=(tk == NT - 1),
                                )

                    # normalize: y_h = y_raw_h / den_h  (den = row 64 of y_ps)
                    rec = norm1.tile([P, HL, QW], f32, tag="rec")
                    for h in range(HL):
                        nc.vector.reciprocal(
                            rec[64:65, h, :], y_ps[h][64:65, :]
                        )
                    dstg = dram.tile([1, HL, QW], f32, tag="dstg")
                    nc.sync.dma_start(dstg[:, :, :], rec[64:65, :, :])
                    rb = norm1.tile([P, HL, QW], f32, tag="rb")
                    dstg_ap = dstg[0, :, :]
                    bcast_src = bass.AP(
                        tensor=dstg_ap.tensor,
                        offset=dstg_ap.offset,
                        ap=[[0, 64], *[list(p) for p in dstg_ap.ap]],
                    )
                    nc.gpsimd.dma_start(rb[0:64, :, :], bcast_src)
                    for h in range(HL):
                        hc, hh = h // 2, h % 2
                        if hh == 0:
                            nc.vector.tensor_mul(
                                out=yt_sb[0:64, hc, tq * QW : (tq + 1) * QW],
                                in0=y_ps[h][0:64, :],
                                in1=rb[0:64, h, :],
                            )
                        else:
                            yst = norm2.tile([64, QW], f32r, tag="yst")
                            nc.vector.tensor_mul(
                                out=yst, in0=y_ps[h][0:64, :], in1=rb[0:64, h, :]
                            )
                            nc.sync.dma_start(
                                yt_sb[64:128, hc, tq * QW : (tq + 1) * QW], yst
                            )

            # ---- stage 3: output projection ----
            with tc.tile_pool(name="ps_o", bufs=3, space="PSUM") as ps_o:
                for tch in range(NT):
                    for co in range(2):
                        po = ps_o.tile([P, QW], f32, tag="o")
                        for dc in range(DC):
                            nc.tensor.matmul(
                                po,
                                yt_sb[:, dc, tch * P : (tch + 1) * P],
                                wo_sb[:, dc, co * QW : (co + 1) * QW],
                                start=(dc == 0),
                                stop=(dc == DC - 1),
                            )
                        o_st = outst.tile([P, QW], f32, tag="o")
                        nc.vector.tensor_copy(o_st, po)
                        nc.sync.dma_start(
                            out_d[tch * P : (tch + 1) * P, co * QW : (co + 1) * QW],
                            o_st,
                        )

    nc.compile()
    return nc


def _get_nc():
    if "nc" not in _CACHE:
        _CACHE["nc"] = _build()
    return _CACHE["nc"]


def _shard_inputs(x_q, x_kv, Wq, bq, Wkv, bkv):
    in_maps = []
    for core in range(NCORES):
        b = core // TPG
        g = core % TPG
        cols = slice(g * CL, (g + 1) * CL)
        in_maps.append(
            {
                "xq": np.ascontiguousarray(x_q[b]),
                "xkv": np.ascontiguousarray(x_kv[b]),
                "wq": np.ascontiguousarray(Wq[:, cols]),
                "wk": np.ascontiguousarray(Wkv[:, :C][:, cols]),
                "wv": np.ascontiguousarray(Wkv[:, C:][:, cols]),
                "wo": None,  # filled by caller (needs Wo)
                "bq": np.ascontiguousarray(bq[cols]),
                "bk": np.ascontiguousarray(bkv[:C][cols]),
            }
        )
    return in_maps


def kernel(x_q, x_kv, Wq, bq, Wkv, bkv, Wo, bo):
    from concourse.bass_utils import run_bass_kernel_spmd

    x_q = np.asarray(x_q, dtype=np.float32)
    x_kv = np.asarray(x_kv, dtype=np.float32)
    Wq = np.asarray(Wq, dtype=np.float32)
    bq = np.asarray(bq, dtype=np.float32)
    Wkv = np.asarray(Wkv, dtype=np.float32)
    bkv = np.asarray(bkv, dtype=np.float32)
    Wo = np.asarray(Wo, dtype=np.float32)
    bo = np.asarray(bo, dtype=np.float32)

    nc = _get_nc()
    in_maps = _shard_inputs(x_q, x_kv, Wq, bq, Wkv, bkv)
    for core in range(NCORES):
        g = core % TPG
        in_maps[core]["wo"] = np.ascontiguousarray(Wo[g * CL : (g + 1) * CL, :])

    res = run_bass_kernel_spmd(nc, in_maps, core_ids=list(range(NCORES)))

    # host-side gather: sum tensor-parallel partials; add exact bias terms
    bias_full = bkv[C:] @ Wo + bo  # v-bias through Wo, plus output bias
    out = np.zeros((B, T, C), dtype=np.float32)
    for core in range(NCORES):
        out[core // TPG] += res.results[core]["out"]
    out += bias_full[None, None, :]
    return out


# revision 15
# speedup vs baseline: 1.0995x; 1.0995x over previous
"""Cross-attention Bass/Tile kernel for Trainium2, sharded over 8 NeuronCores.

Problem (fixed shapes): B=2, T=2048, C=1024, H=16 heads, D=64.
    q = x_q @ Wq + bq;  kv = x_kv @ Wkv + bkv;  k, v = split(kv)
    y = softmax(q k^T / sqrt(D)) v;  out = y @ Wo + bo

Sharding: 8 cores = 2 (batch) x 4 (head groups of 4 heads, 256 channels).
Each core computes its head-group's projections + attention + a partial
output projection (its 256 rows of Wo); the host sums the 4 partials per
batch.  The v-bias and output bias are folded in exactly on the host:
    y = att@(V + 1*bv) = att@V + 1*bv   (att rows sum to 1)
    => out += bv @ Wo + bo              (added once per batch on the host)

On-chip layout (all matmuls fp32r, contraction on partitions):
 - x^T tiles produced by PE transpose (fp32 exact)
 - Q^T, K^T in [d, t] layout: head h at partitions (h%2)*64.., chunk h//2
 - V in natural [t, d] layout (lhsT of the att@V matmul)
 - S^T per (tk,tq) chunk: 2 heads row-packed into one 2-bank psum tile
 - one exp per head-pair on ScalarE, scale=1/8 folded into the affine
 - y^T accumulated in PSUM with 2 heads col-packed per bank;
   denominators via ones-lhsT matmuls col-packed at partitions 0/32/64/96
 - normalization: reciprocal + K=1 broadcast matmul + DVE multiply
 - output projection uses y^T directly as lhsT (no transpose needed)
"""

import numpy as np

B = 2
T = 2048
C = 1024
H = 16
D = 64
NCORES = 8
TPG = 4  # tensor-parallel group size (head groups)
HL = H // TPG  # heads per core = 4
CL = HL * D  # local channels = 256
P = 128

_CACHE = {}


def _build():
    import concourse.bass as bass
    import concourse.tile as tile
    from concourse import bacc, mybir
    from concourse.masks import make_identity

    f32 = mybir.dt.float32
    f32r = mybir.dt.float32r
    Exp = mybir.ActivationFunctionType.Exp

    nc = bacc.Bacc("TRN2", target_bir_lowering=False, debug=False)

    xq_d = nc.dram_tensor("xq", [T, C], f32, kind="ExternalInput")
    xkv_d = nc.dram_tensor("xkv", [T, C], f32, kind="ExternalInput")
    wq_d = nc.dram_tensor("wq", [C, CL], f32, kind="ExternalInput")
    wk_d = nc.dram_tensor("wk", [C, CL], f32, kind="ExternalInput")
    wv_d = nc.dram_tensor("wv", [C, CL], f32, kind="ExternalInput")
    wo_d = nc.dram_tensor("wo", [CL, C], f32, kind="ExternalInput")
    bq_d = nc.dram_tensor("bq", [CL], f32, kind="ExternalInput")
    bk_d = nc.dram_tensor("bk", [CL], f32, kind="ExternalInput")
    out_d = nc.dram_tensor("out", [T, C], f32, kind="ExternalOutput")

    KC = C // P  # 8 contraction chunks for the projections
    NT = T // P  # 16 token chunks of 128
    NQ = 4  # tq chunks of 512
    QW = T // NQ  # 512
    DC = CL // P  # 2 chunks of d_local

    with tile.TileContext(nc) as tc:
        with (
            tc.tile_pool(name="const", bufs=1) as const,
            tc.tile_pool(name="persist", bufs=1) as persist,
            tc.tile_pool(name="xnat", bufs=2) as xnat,
            tc.tile_pool(name="xt", bufs=1) as xtp,
            tc.tile_pool(name="ework", bufs=3) as ework,
            tc.tile_pool(name="norm2", bufs=2) as norm2,
            tc.tile_pool(name="outst", bufs=3) as outst,
        ):
            # ---- constants / weights ----
            ident = const.tile([P, P], f32)
            make_identity(nc, ident)
            ones4_f32 = const.tile([P, HL, 1], f32)
            nc.vector.memset(ones4_f32, 1.0)
            onesb_f32 = const.tile([P, 64], f32)
            nc.vector.memset(onesb_f32, 1.0)
            onesb = const.tile([P, 64], f32r)
            nc.vector.tensor_copy(onesb, onesb_f32)

            wq_sb = const.tile([P, KC, CL], f32r)
            wk_sb = const.tile([P, KC, CL], f32r)
            wv_sb = const.tile([P, KC, CL], f32r)
            wo_sb = const.tile([P, DC, C], f32r)
            for w_sb, w_d in ((wq_sb, wq_d), (wk_sb, wk_d), (wv_sb, wv_d)):
                src = w_d.rearrange("(o p) d -> p o d", p=P).bitcast(f32r)
                for kc in range(KC):
                    nc.gpsimd.dma_start(w_sb[:, kc, :], src[:, kc, :])
            wo_src = wo_d.rearrange("(o p) n -> p o n", p=P).bitcast(f32r)
            for dc in range(DC):
                nc.gpsimd.dma_start(wo_sb[:, dc, :], wo_src[:, dc, :])
            bq_sb = const.tile([P, DC], f32)
            bk_sb = const.tile([P, DC], f32)
            nc.gpsimd.dma_start(bq_sb, bq_d.rearrange("(o p) -> p o", p=P))
            nc.gpsimd.dma_start(bk_sb, bk_d.rearrange("(o p) -> p o", p=P))

            # ---- persistent activations ----
            qt_sb = persist.tile([P, DC, T], f32r)  # Q^T  [d, t]
            kt_sb = persist.tile([P, DC, T], f32r)  # K^T  [d, t]
            v_sb = persist.tile([P, NT, HL, 66], f32r)  # V|1 [t, h, d+1]
            yt_sb = persist.tile([P, DC, T], f32r)  # y^T  [d, t] (normalized)

            # ---- stage 1: transposes + projections, per 512-token chunk ----
            with (
                tc.tile_pool(name="ps_t", bufs=3, space="PSUM") as ps_t,
                tc.tile_pool(name="ps_proj", bufs=3, space="PSUM") as ps_proj,
            ):
                for tq in range(NQ):
                    xq_t = xtp.tile([P, KC, QW], f32r, tag="xqT")
                    xkv_t = xtp.tile([P, KC, QW], f32r, tag="xkvT")
                    for ts_ in range(4):
                        tch = tq * 4 + ts_
                        x_nat = xnat.tile([P, C], f32, tag="xq_nat")
                        kv_nat = xnat.tile([P, C], f32, tag="xkv_nat")
                        for pc in range(4):
                            csl = slice(pc * 256, (pc + 1) * 256)
                            nc.sync.dma_start(
                                x_nat[:, csl], xq_d[tch * P : (tch + 1) * P, csl]
                            )
                            nc.sync.dma_start(
                                kv_nat[:, csl], xkv_d[tch * P : (tch + 1) * P, csl]
                            )
                        for nat, dst_t in ((x_nat, xq_t), (kv_nat, xkv_t)):
                            for grp in range(2):
                                tp = ps_t.tile([P, 4 * P], f32, tag="tp")
                                for cc in range(4):
                                    c = grp * 4 + cc
                                    nc.tensor.transpose(
                                        tp[:, cc * P : (cc + 1) * P],
                                        nat[:, c * P : (c + 1) * P],
                                        ident,
                                    )
                                nc.vector.tensor_copy(
                                    dst_t[:, grp * 4 : (grp + 1) * 4, ts_ * P : (ts_ + 1) * P],
                                    tp.rearrange("p (c t) -> p c t", c=4),
                                )

                    # Q^T and K^T projections: out [d_chunk 128, QW]
                    for src_t, w_sb, b_sb, dst in (
                        (xq_t, wq_sb, bq_sb, qt_sb),
                        (xkv_t, wk_sb, bk_sb, kt_sb),
                    ):
                        for dc in range(DC):
                            pp = ps_proj.tile([P, QW], f32, tag="proj")
                            for c in range(KC):
                                nc.tensor.matmul(
                                    pp,
                                    w_sb[:, c, dc * P : (dc + 1) * P],
                                    src_t[:, c, :],
                                    start=(c == 0),
                                    stop=(c == KC - 1),
                                )
                            nc.vector.tensor_scalar_add(
                                dst[:, dc, tq * QW : (tq + 1) * QW],
                                pp,
                                b_sb[:, dc : dc + 1],
                            )

                    # V projection: out [t chunk 128, CL] (no bias: folded on host)
                    for ts_ in range(4):
                        tch = tq * 4 + ts_
                        pv = ps_proj.tile([P, QW], f32, tag="proj")
                        for c in range(KC):
                            nc.tensor.matmul(
                                pv[:, :CL],
                                xkv_t[:, c, ts_ * P : (ts_ + 1) * P],
                                wv_sb[:, c, :],
                                start=(c == 0),
                                stop=(c == KC - 1),
                            )
                        nc.vector.tensor_copy(
                            v_sb[:, tch, :, 0:64],
                            pv[:, :CL].rearrange("p (h d) -> p h d", h=HL),
                        )
                        nc.vector.tensor_copy(v_sb[:, tch, :, 64:65], ones4_f32)

            # ---- stage 2: attention, per tq chunk of 512 ----
            with (
                tc.tile_pool(name="ps_s", bufs=2, space="PSUM") as ps_s,
                tc.tile_pool(name="ps_y", bufs=4, space="PSUM") as ps_y,
            ):
                for tq in range(NQ):
                    y_ps = [ps_y.tile([65, QW], f32, tag="y", name=f"y_ps{i}") for i in range(HL)]
                    for tk in range(NT):
                        for hc in range(DC):  # head pair (2*hc, 2*hc+1)
                            sp = ps_s.tile([P, 2 * QW], f32, tag="s")
                            for hh in range(2):
                                nc.tensor.matmul(
                                    sp[:, hh * QW : (hh + 1) * QW],
                                    kt_sb[
                                        hh * 64 : (hh + 1) * 64,
                                        hc,
                                        tk * P : (tk + 1) * P,
                                    ],
                                    qt_sb[
                                        hh * 64 : (hh + 1) * 64,
                                        hc,
                                        tq * QW : (tq + 1) * QW,
                                    ],
                                    start=True,
                                    stop=True,
                                    tile_position=(hh * 64, 0),
                                )
                            e2 = ework.tile([P, 2 * QW], f32r, tag="e")
                            nc.scalar.activation(e2, sp, Exp, scale=0.125)
                            for hh in range(2):
                                h = 2 * hc + hh
                                nc.tensor.matmul(
                                    y_ps[h],
                                    v_sb[:, tk, h, :65],
                                    e2[:, hh * QW : (hh + 1) * QW],
                                    start=(tk == 0),
                                    stop=(tk == NT - 1),
                                )

                    # normalize: y_h = y_raw_h / den_h  (den = row 64 of y_ps)
                    rec = norm2.tile([P, HL, QW], f32, tag="rec")
                    for h in range(HL):
                        nc.vector.reciprocal(rec[64:65, h, :], y_ps[h][64:65, :])
                    recr = norm2.tile([P, HL, QW], f32r, tag="recr")
                    nc.vector.tensor_copy(recr[64:65, :, :], rec[64:65, :, :])
                    for hc in range(DC):
                        rbp = ps_s.tile([P, 2 * QW], f32, tag="s", name=f"rb{hc}")
                        for hh in range(2):
                            h = 2 * hc + hh
                            nc.tensor.matmul(
                                rbp[0:64, hh * QW : (hh + 1) * QW],
                                onesb[64:65, :],
                                recr[64:65, h, :],
                                start=True,
                                stop=True,
                                tile_position=(64, 0),
                                skip_group_check=True,
                            )
                        rbs = norm2.tile([P, 2 * QW], f32, tag="rbs")
                        nc.vector.tensor_copy(rbs[0:64, :], rbp[0:64, :])
                        for hh in range(2):
                            h = 2 * hc + hh
                            rb_h = rbs[0:64, hh * QW : (hh + 1) * QW]
                            if hh == 0:
                                nc.vector.tensor_mul(
                                    out=yt_sb[0:64, hc, tq * QW : (tq + 1) * QW],
                                    in0=y_ps[h][0:64, :],
                                    in1=rb_h,
                                )
                            else:
                                yst = norm2.tile([64, QW], f32r, tag="yst")
                                nc.vector.tensor_mul(
                                    out=yst, in0=y_ps[h][0:64, :], in1=rb_h
                                )
                                nc.sync.dma_start(
                                    yt_sb[64:128, hc, tq * QW : (tq + 1) * QW], yst
                                )

            # ---- stage 3: output projection ----
            with tc.tile_pool(name="ps_o", bufs=3, space="PSUM") as ps_o:
                for tch in range(NT):
                    for co in range(2):
                        po = ps_o.tile([P, QW], f32, tag="o")
                        for dc in range(DC):
                            nc.tensor.matmul(
                                po,
                                yt_sb[:, dc, tch * P : (tch + 1) * P],
                                wo_sb[:, dc, co * QW : (co + 1) * QW],
                                start=(dc == 0),
                                stop=(dc == DC - 1),
                            )
                        o_st = outst.tile([P, QW], f32, tag="o")
                        nc.vector.tensor_copy(o_st, po)
                        nc.sync.dma_start(
                            out_d[tch * P : (tch + 1) * P, co * QW : (co + 1) * QW],
                            o_st,
                        )

    nc.compile()
    return nc


def _get_nc():
    if "nc" not in _CACHE:
        _CACHE["nc"] = _build()
    return _CACHE["nc"]


def _shard_inputs(x_q, x_kv, Wq, bq, Wkv, bkv):
    in_maps = []
    for core in range(NCORES):
        b = core // TPG
        g = core % TPG
        cols = slice(g * CL, (g + 1) * CL)
        in_maps.append(
            {
                "xq": np.ascontiguousarray(x_q[b]),
                "xkv": np.ascontiguousarray(x_kv[b]),
                "wq": np.ascontiguousarray(Wq[:, cols]),
                "wk": np.ascontiguousarray(Wkv[:, :C][:, cols]),
                "wv": np.ascontiguousarray(Wkv[:, C:][:, cols]),
                "wo": None,  # filled by caller (needs Wo)
                "bq": np.ascontiguousarray(bq[cols]),
                "bk": np.ascontiguousarray(bkv[:C][cols]),
            }
        )
    return in_maps


def kernel(x_q, x_kv, Wq, bq, Wkv, bkv, Wo, bo):
    from concourse.bass_utils import run_bass_kernel_spmd

    x_q = np.asarray(x_q, dtype=np.float32)
    x_kv = np.asarray(x_kv, dtype=np.float32)
    Wq = np.asarray(Wq, dtype=np.float32)
    bq = np.asarray(bq, dtype=np.float32)
    Wkv = np.asarray(Wkv, dtype=np.float32)
    bkv = np.asarray(bkv, dtype=np.float32)
    Wo = np.asarray(Wo, dtype=np.float32)
    bo = np.asarray(bo, dtype=np.float32)

    nc = _get_nc()
    in_maps = _shard_inputs(x_q, x_kv, Wq, bq, Wkv, bkv)
    for core in range(NCORES):
        g = core % TPG
        in_maps[core]["wo"] = np.ascontiguousarray(Wo[g * CL : (g + 1) * CL, :])

    res = run_bass_kernel_spmd(nc, in_maps, core_ids=list(range(NCORES)))

    # host-side gather: sum tensor-parallel partials; add exact bias terms
    bias_full = bkv[C:] @ Wo + bo  # v-bias through Wo, plus output bias
    out = np.zeros((B, T, C), dtype=np.float32)
    for core in range(NCORES):
        out[core // TPG] += res.results[core]["out"]
    out += bias_full[None, None, :]
    return out
